# revision 1
# baseline (speedup 1.0000x reference)
"""Chamfer loss Bass/Tile kernel for Trainium2 (8 NeuronCores, SPMD).

Problem: x, y [B=32, D=128, N=2048] f32, mask [B, N] bool (shared by x and y).
  d[b,i,j] = ||x_i - y_j||^2;  loss = mean_b( sum_j min_i d + sum_i min_j d )
  (mins/sums over valid entries only).

Strategy (data-parallel over batch, 4 batches per core):
  - Gram tiles G[i,j] = x_i . y_j via fp32r matmuls (contraction = D = 128).
  - A K=2 "prefill" matmul first writes -y2m[j]/2 into PSUM
    (y2m = ||y_j||^2 + (1-m_j)*BIG); the main matmul accumulates, so
    PSUM = G - y2m/2. The ACT evacuation T = Identity(2*PSUM + bias) with
    per-partition bias -x2m[i] yields T = 2G - x2m - y2m = -d, with masked
    rows/cols pushed to -BIG so they never win a max.
  - Per [128 i x 1024 j] PSUM group: col-wise max via DVE tensor_scalar
    accum_out (-> min_j d per i) and row-wise running max via DVE
    tensor_tensor (-> min_i d per j, finished by PE transposes + a reduce).
  - x2 in cols layout [128,16] via 16 small matmuls (xsq-chunk x -1 vector)
    rides the ACT-evac bias; y2 row via 0.5-weighted ones-matmuls + DVE
    copies feeds a K=2 fp32r PSUM-prefill matmul.
  - Host sums the 8 per-core [128, 2] partials and divides by B.
"""

import numpy as np
from contextlib import ExitStack

import concourse.mybir as mybir
import concourse.tile as tile
from concourse import bacc
from concourse.masks import make_identity

F32 = mybir.dt.float32
F32R = mybir.dt.float32r
BF16 = mybir.dt.bfloat16
AX = mybir.AxisListType
OP = mybir.AluOpType
ACTF = mybir.ActivationFunctionType

B, D, N = 32, 128, 2048
CORES = 8
BPC = B // CORES          # batches per core
JCH, NJ = 512, N // 512   # j-chunk size / count (matmul granularity)
GW, NG = 1024, N // 1024  # j-group size / count (evac/reduce granularity)
ICH, NI = 128, N // 128   # i-chunk size / count
BIG = 1e9


def build_nc():
    nc = bacc.Bacc("TRN2", target_bir_lowering=False, debug=False)
    x_d = nc.dram_tensor("x", [BPC, D, N], F32, kind="ExternalInput").ap()
    y_d = nc.dram_tensor("y", [BPC, D, N], F32, kind="ExternalInput").ap()
    bigh_d = nc.dram_tensor("bigh", [BPC, 1, N], F32, kind="ExternalInput").ap()
    neg1_d = nc.dram_tensor("neg1", [2, N], F32, kind="ExternalInput").ap()
    mcols_d = nc.dram_tensor("mcols", [BPC, D, NI], F32, kind="ExternalInput").ap()
    out_d = nc.dram_tensor("out", [D, 2], F32, kind="ExternalOutput").ap()

    with tile.TileContext(nc) as tc:
        with ExitStack() as ctx:
            _emit(ctx, tc, out_d, x_d, y_d, bigh_d, neg1_d, mcols_d)
    nc.compile()
    return nc


def _emit(ctx, tc, out_d, x_d, y_d, bigh_d, neg1_d, mcols_d):
    nc = tc.nc
    io = ctx.enter_context(tc.tile_pool(name="io", bufs=2))  # xs/ys
    sq = ctx.enter_context(tc.tile_pool(name="sq", bufs=1))
    pre = ctx.enter_context(tc.tile_pool(name="pre", bufs=1))
    tp = ctx.enter_context(tc.tile_pool(name="tp", bufs=8))
    rp = ctx.enter_context(tc.tile_pool(name="rp", bufs=2))
    small = ctx.enter_context(tc.tile_pool(name="small", bufs=2))
    accp = ctx.enter_context(tc.tile_pool(name="accp", bufs=1))
    prep = ctx.enter_context(tc.tile_pool(name="prep", bufs=2))
    pp = ctx.enter_context(tc.tile_pool(name="pp", bufs=2, space="PSUM"))
    pp2 = ctx.enter_context(tc.tile_pool(name="pp2", bufs=1, space="PSUM"))
    ppc = ctx.enter_context(tc.tile_pool(name="ppc", bufs=2, space="PSUM"))
    prt = ctx.enter_context(tc.tile_pool(name="prt", bufs=1, space="PSUM"))

    acc = accp.tile([D, 2], F32)
    nc.vector.memset(acc[:], 0.0)

    halves = pre.tile([D, 1], F32, tag="halves")      # 0.5 (y2 row matmuls)
    nc.gpsimd.memset(halves[:], 0.5)
    negones = pre.tile([D, 1], F32, tag="negones")    # -1  (x2 cols matmuls)
    nc.gpsimd.memset(negones[:], -1.0)
    # prefill lhsT: two rows of -1 (pairs with [y2h; bigh] rows of pre_rhs),
    # shipped from host (fp32r matmul inputs must not be engine-written).
    neg1x2 = pre.tile([2, N], F32R, tag="neg1x2")
    nc.sync.dma_start(out=neg1x2[:], in_=neg1_d.bitcast(F32R))
    ident = pre.tile([ICH, ICH], BF16, tag="ident")
    make_identity(nc, ident[:])

    def emit_load(b):
        pre_rhs = prep.tile([2, N], F32R, tag="prhs", name=f"prhs{b}")
        mcols = small.tile([D, NI], F32, tag="mcols", name=f"mcols{b}")
        nc.gpsimd.dma_start(out=mcols[:], in_=mcols_d[b])
        nc.gpsimd.dma_start(out=pre_rhs[1:2, :], in_=bigh_d[b].bitcast(F32R))
        xs = io.tile([D, N], F32R, tag="xs", name=f"xs{b}")
        ys = io.tile([D, N], F32R, tag="ys", name=f"ys{b}")
        xsq = sq.tile([D, N], F32, tag="xsq", name=f"xsq{b}")
        ysq = sq.tile([D, N], F32, tag="ysq", name=f"ysq{b}")
        for c in range(NJ):
            cs = slice(c * JCH, (c + 1) * JCH)
            nc.sync.dma_start(out=ys[:, cs], in_=y_d[b][:, cs].bitcast(F32R))
            nc.sync.dma_start(out=xs[:, cs], in_=x_d[b][:, cs].bitcast(F32R))
            nc.gpsimd.tensor_tensor(ysq[:, cs], ys[:, cs], ys[:, cs], op=OP.mult)
            nc.gpsimd.tensor_tensor(xsq[:, cs], xs[:, cs], xs[:, cs], op=OP.mult)
        return {"pre_rhs": pre_rhs, "mcols": mcols, "xs": xs, "ys": ys,
                "xsq": xsq, "ysq": ysq}

    def emit_norms(b, st):
        pre_rhs, mcols, xsq, ysq = st["pre_rhs"], st["mcols"], st["xsq"], st["ysq"]
        y2row = small.tile([1, N], F32, tag="y2row", name=f"y2row{b}")
        pscols = ppc.tile([D, NI], F32, tag="pscols", name=f"pscols{b}")
        for c in range(NJ):
            cs = slice(c * JCH, (c + 1) * JCH)
            py = pp2.tile([1, JCH], F32, tag="prow")
            nc.tensor.matmul(py[:], lhsT=halves[:], rhs=ysq[:, cs],
                             start=True, stop=True)
            if c % 2 == 0:
                nc.scalar.activation(y2row[0:1, cs], py[:], ACTF.Copy,
                                     bias=0.0, scale=1.0)
            else:
                nc.vector.tensor_scalar(y2row[0:1, cs], py[:], 1.0, None,
                                        op0=OP.mult)
            nc.gpsimd.dma_start(out=pre_rhs[0:1, cs],
                                in_=y2row[0:1, cs].bitcast(F32R))
            for t in range(4 * c, 4 * c + 4):
                nc.tensor.matmul(pscols[:, t:t + 1],
                                 lhsT=xsq[:, t * ICH:(t + 1) * ICH],
                                 rhs=negones[:],
                                 start=True, stop=True)
        bigm = small.tile([D, NI], F32, tag="bigm", name=f"bigm{b}")
        nc.vector.tensor_scalar(bigm[:], mcols[:], 1.0, BIG,
                                op0=OP.subtract, op1=OP.mult)
        x2neg = small.tile([D, NI], F32, tag="x2neg", name=f"x2neg{b}")
        for c in range(NJ):
            cc = slice(4 * c, 4 * c + 4)
            nc.vector.tensor_tensor(x2neg[:, cc], pscols[:, cc], bigm[:, cc],
                                    op=OP.add)
        st["x2neg"] = x2neg

    st = emit_load(0)
    emit_norms(0, st)
    nxt = None
    for b in range(BPC):
        xs, ys = st["xs"], st["ys"]
        x2neg, mcols, pre_rhs = st["x2neg"], st["mcols"], st["pre_rhs"]

        R = rp.tile([D, N], BF16, tag="R")            # running max over i-chunks
        cmax = small.tile([D, NI * NG], F32, tag="cmax")
        rtc = small.tile([D, NI], F32, tag="rtc")
        for jg in range(NG):
            for ic in range(NI):
                ps = pp.tile([D, GW], F32, tag="ps")
                for h in range(GW // JCH):
                    j0 = jg * GW + h * JCH
                    psl = ps[:, h * JCH:(h + 1) * JCH]
                    nc.tensor.matmul(
                        psl,
                        lhsT=neg1x2[:, ic * ICH:(ic + 1) * ICH],
                        rhs=pre_rhs[:, j0:j0 + JCH],
                        start=True, stop=False)
                    nc.tensor.matmul(
                        psl,
                        lhsT=xs[:, ic * ICH:(ic + 1) * ICH],
                        rhs=ys[:, j0:j0 + JCH],
                        start=False, stop=True)
                t16 = tp.tile([D, GW], BF16, tag="t16")
                nc.scalar.activation(t16[:], ps[:], ACTF.Identity,
                                     bias=x2neg[:, ic:ic + 1], scale=2.0)
                # col path: max over j within this group -> cmax column
                scr = tp.tile([D, GW], BF16, tag="scr")
                k = ic * NG + jg
                nc.vector.tensor_scalar(scr[:], t16[:], 0.0, None,
                                        op0=OP.add, op1=OP.max,
                                        accum_out=cmax[:, k:k + 1])
                # row path: running elementwise max over i-chunks
                rsl = R[:, jg * GW:(jg + 1) * GW]
                nc.vector.tensor_tensor(rsl, t16[:], t16[:] if ic == 0 else rsl,
                                        op=OP.max)
                if jg == 1 and ic == 11 and b + 1 < BPC:
                    emit_norms(b + 1, nxt)
            if jg == 0 and b + 1 < BPC:
                nxt = emit_load(b + 1)

        # x_dist partial: -sum_j m_j * max_i T[i,j]
        # (max over partitions: PE-transpose each 128-block into PSUM, reduce)
        NT = GW // ICH
        for jg in range(NG):
            RT = prt.tile([D, GW], BF16, tag="RT")
            for t in range(NT):
                tt = jg * NT + t
                nc.tensor.transpose(RT[:, t * ICH:(t + 1) * ICH],
                                    R[:, tt * ICH:(tt + 1) * ICH], ident[:])
            nc.vector.tensor_reduce(rtc[:, jg * NT:(jg + 1) * NT],
                                    RT[:].rearrange("p (t q) -> p t q", q=ICH),
                                    axis=AX.X, op=OP.max)
        tX = small.tile([D, NI], F32, tag="tX")
        nc.vector.tensor_tensor(tX[:], rtc[:], mcols[:], op=OP.mult)
        sX = small.tile([D, 1], F32, tag="sX")
        nc.vector.tensor_reduce(sX[:], tX[:], axis=AX.X, op=OP.add)
        nc.vector.tensor_tensor(acc[:, 0:1], acc[:, 0:1], sX[:], op=OP.subtract)

        # y_dist partial: -sum_i m_i * max_j T[i,j]
        cm16 = small.tile([D, NI], F32, tag="cm16")
        nc.vector.tensor_reduce(cm16[:], cmax[:].rearrange("p (i j) -> p i j", j=NG),
                                axis=AX.X, op=OP.max)
        tY = small.tile([D, NI], F32, tag="tY")
        nc.vector.tensor_tensor(tY[:], cm16[:], mcols[:], op=OP.mult)
        sY = small.tile([D, 1], F32, tag="sY")
        nc.vector.tensor_reduce(sY[:], tY[:], axis=AX.X, op=OP.add)
        nc.vector.tensor_tensor(acc[:, 1:2], acc[:, 1:2], sY[:], op=OP.subtract)
        if nxt is not None:
            st = nxt
            nxt = None

    nc.sync.dma_start(out=out_d, in_=acc[:])


def prepare_in_maps(x, y, mask):
    mf = mask.astype(np.float32)                       # [B, N]
    bigh = ((1.0 - mf) * (BIG / 2)).astype(np.float32)
    mcols = np.ascontiguousarray(
        mf.reshape(B, NI, ICH).transpose(0, 2, 1))     # [B, 128, 16]
    neg1 = np.full((2, N), -1.0, dtype=np.float32)
    in_maps = []
    for c in range(CORES):
        s = slice(c * BPC, (c + 1) * BPC)
        in_maps.append({
            "x": np.ascontiguousarray(x[s]),
            "y": np.ascontiguousarray(y[s]),
            "bigh": np.ascontiguousarray(bigh[s][:, None, :]),
            "neg1": neg1,
            "mcols": np.ascontiguousarray(mcols[s]),
        })
    return in_maps


def finish(per_core_outs):
    """per_core_outs: list of 8 arrays [128, 2] -> scalar loss."""
    total = 0.0
    for o in per_core_outs:
        total += np.asarray(o, dtype=np.float64).sum()
    return np.float32(total / B)


_NC = None


def kernel(x, y, mask):
    global _NC
    if _NC is None:
        _NC = build_nc()
    from concourse.bass_utils import run_bass_kernel_spmd
    in_maps = prepare_in_maps(np.asarray(x), np.asarray(y), np.asarray(mask))
    res = run_bass_kernel_spmd(_NC, in_maps, list(range(CORES)))
    return finish([res.results[c]["out"] for c in range(CORES)])



# revision 3
# speedup vs baseline: 1.4297x; 1.4297x over previous
"""Chamfer loss Bass/Tile kernel for Trainium2 (8 NeuronCores, SPMD).

Problem: x, y [B=32, D=128, N=2048] f32, mask [B, N] bool (shared by x and y).
  d[b,i,j] = ||x_i - y_j||^2;  loss = mean_b( sum_j min_i d + sum_i min_j d )
  (mins/sums over valid entries only).

Strategy (data-parallel over batch, 4 batches per core):
  - Work in S = -d/2 = G - x2/2 - y2/2 form. PSUM group [128 x 1024] gets
    S directly: an fp8e4 DoubleRow "prefill" matmul (K=2, hi/lo split rows,
    0.5 cycles/col) injects -x2m/2 (per-partition) and -y2m/2 (per-column)
    including +BIG masking, then bf16 main matmuls accumulate G on top.
    No per-tile bias work remains for the vector engines.
  - Evac+col: ~70% of groups evacuate PSUM->bf16 SBUF via ACT (Copy) and
    run a DVE tensor_scalar (4x mode) whose accum_out gives max_j S per
    partition; ~30% of groups use a single Pool tensor_scalar that does
    evac + accum in one op. min_j d = -2 max_j S.
  - Row path: running elementwise max over the 16 i-chunks (TT max, split
    DVE/Pool); the ic==0 evac writes straight into R. Finished per batch by
    PE transposes + a grouped tensor_reduce -> max_i S per j.
  - Masked sums against host-shipped mask cols; host scales by -2/B.
"""

import numpy as np
import ml_dtypes
from contextlib import ExitStack

import concourse.mybir as mybir
import concourse.tile as tile
from concourse import bacc
from concourse.masks import make_identity

F32 = mybir.dt.float32
BF16 = mybir.dt.bfloat16
F8E4 = mybir.dt.float8e4
AX = mybir.AxisListType
OP = mybir.AluOpType
ACTF = mybir.ActivationFunctionType
DR = mybir.MatmulPerfMode.DoubleRow

B, D, N = 32, 128, 2048
CORES = 8
BPC = B // CORES          # batches per core
ICH, NI = 128, N // 128   # i-chunk size / count
GW, NG = 1024, N // 1024  # j-group width / count (evac granularity)
MMW = 512                 # matmul width (one PSUM bank)
MBIG = 288.0              # mask push (d shifted by 2*MBIG per masked side)

# group index gi = ic*NG+jg in 0..31; these evac via Pool (fused col accum),
# the rest via ACT + DVE col pass.
POOL_EVAC = frozenset(gi for gi in range(NI * NG) if gi % 10 in (3, 6, 9))
# row TT ops (ic>=1): sent to Pool when (ic*NG+jg) % 7 == 2, else DVE.
def _row_on_pool(gi):
    return gi % 7 == 2


def build_nc():
    nc = bacc.Bacc("TRN2", target_bir_lowering=False, debug=False)
    x_d = nc.dram_tensor("x", [BPC, D, N], BF16, kind="ExternalInput").ap()
    y_d = nc.dram_tensor("y", [BPC, D, N], BF16, kind="ExternalInput").ap()
    pfl_d = nc.dram_tensor("pfl", [BPC, 2, 2, N], F8E4, kind="ExternalInput").ap()
    pfr_d = nc.dram_tensor("pfr", [BPC, 2, 2, N], F8E4, kind="ExternalInput").ap()
    mcols_d = nc.dram_tensor("mcols", [BPC, D, NI], F32, kind="ExternalInput").ap()
    out_d = nc.dram_tensor("out", [D, 2], F32, kind="ExternalOutput").ap()

    with tile.TileContext(nc) as tc:
        with ExitStack() as ctx:
            _emit(ctx, tc, out_d, x_d, y_d, pfl_d, pfr_d, mcols_d)
    nc.compile()
    return nc


def _emit(ctx, tc, out_d, x_d, y_d, pfl_d, pfr_d, mcols_d):
    nc = tc.nc
    io = ctx.enter_context(tc.tile_pool(name="io", bufs=2))
    pf = ctx.enter_context(tc.tile_pool(name="pf", bufs=2))
    bp = ctx.enter_context(tc.tile_pool(name="bp", bufs=6))
    rp = ctx.enter_context(tc.tile_pool(name="rp", bufs=2))
    scp = ctx.enter_context(tc.tile_pool(name="scp", bufs=2))
    small = ctx.enter_context(tc.tile_pool(name="small", bufs=2))
    accp = ctx.enter_context(tc.tile_pool(name="accp", bufs=1))
    pre = ctx.enter_context(tc.tile_pool(name="pre", bufs=1))
    pp = ctx.enter_context(tc.tile_pool(name="pp", bufs=3, space="PSUM"))
    prt = ctx.enter_context(tc.tile_pool(name="prt", bufs=1, space="PSUM"))

    acc = accp.tile([D, 2], F32)
    nc.vector.memset(acc[:], 0.0)
    ident = pre.tile([ICH, ICH], BF16, tag="ident")
    make_identity(nc, ident[:])

    def emit_load(b):
        st = {}
        st["xs"] = io.tile([D, N], BF16, tag="xs", name=f"xs{b}")
        st["ys"] = io.tile([D, N], BF16, tag="ys", name=f"ys{b}")
        st["pfl"] = pf.tile([2, 2, N], F8E4, tag="pfl", name=f"pfl{b}")
        st["pfr"] = pf.tile([2, 2, N], F8E4, tag="pfr", name=f"pfr{b}")
        st["mcols"] = small.tile([D, NI], F32, tag="mcols", name=f"mcols{b}")
        nc.sync.dma_start(out=st["pfl"][:], in_=pfl_d[b])
        nc.sync.dma_start(out=st["pfr"][:], in_=pfr_d[b])
        nc.sync.dma_start(out=st["mcols"][:], in_=mcols_d[b])
        nc.sync.dma_start(out=st["xs"][:], in_=x_d[b])
        nc.sync.dma_start(out=st["ys"][:], in_=y_d[b])
        return st

    st = emit_load(0)
    nxt = None
    for b in range(BPC):
        xs, ys, pfl, pfr, mcols = (st["xs"], st["ys"], st["pfl"], st["pfr"],
                                   st["mcols"])
        R = rp.tile([D, N], BF16, tag="R", name=f"R{b}")
        cm = small.tile([D, NI * NG], F32, tag="cm", name=f"cm{b}")
        for ic in range(NI):
            lsl = slice(ic * ICH, (ic + 1) * ICH)
            for jg in range(NG):
                gi = ic * NG + jg
                ps = pp.tile([D, GW], F32, tag="ps")
                for h in range(GW // MMW):
                    j0 = jg * GW + h * MMW
                    psl = ps[:, h * MMW:(h + 1) * MMW]
                    nc.tensor.matmul(psl, lhsT=pfl[:, :, lsl],
                                     rhs=pfr[:, :, j0:j0 + MMW],
                                     start=True, stop=False, perf_mode=DR)
                    nc.tensor.matmul(psl, lhsT=xs[:, lsl],
                                     rhs=ys[:, j0:j0 + MMW],
                                     start=False, stop=True)
                rsl = R[:, jg * GW:(jg + 1) * GW]
                if ic == 0:
                    bdst = rsl
                else:
                    bdst = bp.tile([D, GW], BF16, tag="bt")
                cma = cm[:, gi:gi + 1]
                if gi in POOL_EVAC:
                    nc.gpsimd.tensor_scalar(bdst, ps[:], 0.0, None,
                                            op0=OP.add, op1=OP.max,
                                            accum_out=cma)
                else:
                    nc.scalar.activation(bdst, ps[:], ACTF.Copy,
                                         bias=0.0, scale=1.0)
                    scr = scp.tile([D, GW], BF16, tag="scr")
                    nc.vector.tensor_scalar(scr[:], bdst, 0.0, None,
                                            op0=OP.add, op1=OP.max,
                                            accum_out=cma)
                if ic > 0:
                    eng = nc.gpsimd if _row_on_pool(gi) else nc.vector
                    eng.tensor_tensor(rsl, bdst, rsl, op=OP.max)
            if ic == 2 and b + 1 < BPC:
                nxt = emit_load(b + 1)

        # row finish: max over partitions via PE transposes + grouped reduce
        rt = prt.tile([D, N], BF16, tag="rt")
        for t in range(NI):
            nc.tensor.transpose(rt[:, t * ICH:(t + 1) * ICH],
                                R[:, t * ICH:(t + 1) * ICH], ident[:])
        rr = small.tile([D, NI], F32, tag="rr")
        nc.vector.tensor_reduce(rr[:], rt[:].rearrange("p (t q) -> p t q", q=ICH),
                                axis=AX.X, op=OP.max)
        cmf = small.tile([D, NI], F32, tag="cmf")
        nc.vector.tensor_reduce(cmf[:], cm[:].rearrange("p (i g) -> p i g", g=NG),
                                axis=AX.X, op=OP.max)
        tX = small.tile([D, NI], F32, tag="tX")
        nc.vector.tensor_tensor(tX[:], rr[:], mcols[:], op=OP.mult)
        tY = small.tile([D, NI], F32, tag="tY")
        nc.vector.tensor_tensor(tY[:], cmf[:], mcols[:], op=OP.mult)
        sX = small.tile([D, 1], F32, tag="sX")
        nc.vector.tensor_reduce(sX[:], tX[:], axis=AX.X, op=OP.add)
        sY = small.tile([D, 1], F32, tag="sY")
        nc.vector.tensor_reduce(sY[:], tY[:], axis=AX.X, op=OP.add)
        nc.vector.tensor_tensor(acc[:, 0:1], acc[:, 0:1], sX[:], op=OP.add)
        nc.vector.tensor_tensor(acc[:, 1:2], acc[:, 1:2], sY[:], op=OP.add)
        if nxt is not None:
            st = nxt
            nxt = None

    nc.sync.dma_start(out=out_d, in_=acc[:])


def _hilo_e4m3(v):
    """Split v >= 0 into hi+lo fp8e4m3 (clipped to the 240 max)."""
    hi = np.minimum(v, 240.0).astype(ml_dtypes.float8_e4m3)
    lo = (v - hi.astype(np.float64)).astype(ml_dtypes.float8_e4m3)
    return hi, lo


def prepare_in_maps(x, y, mask):
    xb = np.asarray(x).astype(ml_dtypes.bfloat16)          # [B, D, N]
    yb = np.asarray(y).astype(ml_dtypes.bfloat16)
    mf = np.asarray(mask).astype(np.float64)               # [B, N]
    x2 = (xb.astype(np.float64) ** 2).sum(axis=1)          # [B, N]
    y2 = (yb.astype(np.float64) ** 2).sum(axis=1)
    vx = x2 / 2 + MBIG * (1.0 - mf)
    vy = y2 / 2 + MBIG * (1.0 - mf)
    xhi, xlo = _hilo_e4m3(vx)
    yhi, ylo = _hilo_e4m3(vy)
    pfl = np.empty((B, 2, 2, N), dtype=ml_dtypes.float8_e4m3)
    pfr = np.empty((B, 2, 2, N), dtype=ml_dtypes.float8_e4m3)
    pfl[:, 0, :, :] = -1.0
    pfl[:, 1, 0, :] = xhi
    pfl[:, 1, 1, :] = xlo
    pfr[:, 0, 0, :] = yhi
    pfr[:, 0, 1, :] = ylo
    pfr[:, 1, :, :] = -1.0
    mcols = np.ascontiguousarray(
        mf.astype(np.float32).reshape(B, NI, ICH).transpose(0, 2, 1))
    in_maps = []
    for c in range(CORES):
        s = slice(c * BPC, (c + 1) * BPC)
        in_maps.append({
            "x": np.ascontiguousarray(xb[s]),
            "y": np.ascontiguousarray(yb[s]),
            "pfl": np.ascontiguousarray(pfl[s]),
            "pfr": np.ascontiguousarray(pfr[s]),
            "mcols": np.ascontiguousarray(mcols[s]),
        })
    return in_maps


def finish(per_core_outs):
    """per_core_outs: list of 8 arrays [128, 2] -> scalar loss."""
    total = 0.0
    for o in per_core_outs:
        total += np.asarray(o, dtype=np.float64).sum()
    return np.float32(-2.0 * total / B)


_NC = None


def kernel(x, y, mask):
    global _NC
    if _NC is None:
        _NC = build_nc()
    from concourse.bass_utils import run_bass_kernel_spmd
    in_maps = prepare_in_maps(np.asarray(x), np.asarray(y), np.asarray(mask))
    res = run_bass_kernel_spmd(_NC, in_maps, list(range(CORES)))
    return finish([res.results[c]["out"] for c in range(CORES)])
